# revision 11
# baseline (speedup 1.0000x reference)
"""Trainium2 Bass kernel for nn_Convolution_77111842832763.

3D conv 5x5x5 SAME, 64->64 channels, input [2,40,40,40,64] fp32, plus an
irrep-wise linear self-connection (folded into the conv's center tap).

Strategy (8 NeuronCores, data-parallel):
  - Shard: core = batch(2) x x-chunk(4); each core computes a [10,40,40,64]
    output slab from a zero-padded [14,44,44,64] input slab (halo 2).
  - Host builds the 5^3 x 64 x 64 tensor-product kernel exactly (float64),
    folds the self-connection into the center tap, and packs per-tap weight
    blocks; weights/slabs are cast to fp16 on host (device matmuls run fp16
    with fp32 PSUM accumulation; measured end-to-end rel-err ~2.9e-4).
  - Device: TWO channel-major slab copies in SBUF [128, 14*44*44]:
      slab_z: partitions 0-63 = slab, 64-127 = slab shifted +1 z-voxel
      slab_y: partitions 0-63 = slab, 64-127 = slab shifted +1 y-row
    K=128 packing: the 125 taps become 63 matmul units per output tile:
      50 z-pair units  (dx,dy, dz in {(0,1),(2,3)})        -> slab_z
      10 y-pair units  (dx, dz=4 slice, dy in {(0,1),(2,3)}) -> slab_y
       3 x-pair units  (dy=4,dz=4 column, dx in {(0,1),(2,3),(4,zero)}) -> slab_x
  - Units alternate between PE column groups 0-63/64-127 (2x column
    tiling) accumulating into psum[0:64]/psum[64:128]; the two partial
    sums are DMA'd out separately and added on host.
  - Output tile = one x-plane quarter: 10 y-rows x 40 z = 400 voxels
    (moving free dim 400, one PSUM bank).  40 tiles per core.
  - DMA: z/x-copy planes on the SP HWDGE ring, weights/y-planes/outputs
    on the ACT ring (parallel rings), each ring ordered by consumption
    deadline; z-plane 0 is split across both rings so the first matmul's
    gate lands ~2x sooner.  A JSON post-pass splits multi-wait
    instructions (this walrus build allows one sync wait per instruction).
"""

import functools
import json
import math
from contextlib import ExitStack

import numpy as np

import concourse.bass as bass
import concourse.mybir as mybir
import concourse.tile as tile
from concourse.bass_utils import run_bass_kernel_spmd

MUL = 16
DIM = 64
NB = 8
PX, PY, PZ = 14, 44, 44          # padded slab dims
PLANE = PY * PZ                   # 1936
SLAB = PX * PLANE                 # 27104
SLAB_PAD = SLAB + 48              # tail zeros so the +1z and +44y views stay in-bounds
# unit list: ("z", dx, dy, zg) -> taps (dx,dy,2*zg)+(dx,dy,2*zg+1) via slab_z
#            ("y", dx, yg)     -> taps (dx,2*yg,4)+(dx,2*yg+1,4)   via slab_y
#            ("x", xg)         -> taps (2*xg,4,4)+(2*xg+1,4,4)     via slab_x
# The weight table holds all 68 unit blocks; every output plane runs the
# 63-unit scheme (the input DMA order streams the x-copy planes early
# enough that even plane 0's three x-pair units arrive in time).
UNITS = [("z", dx, dy, zg) for dx in range(5) for dy in range(5) for zg in range(2)]
UNITS += [("y", dx, yg, 0) for dx in range(5) for yg in range(3)]
UNITS += [("x", xg, 0, 0) for xg in range(3)]
NU = len(UNITS)                   # 68 weight blocks
U63 = list(range(50)) + [50 + 3 * dx + yg for dx in range(5) for yg in range(2)] \
    + [65, 66, 67]                                         # z50 + y10 + x3


def _split_sync_waits_json(raw: bytes) -> bytes:
    """Hoist all but the last sync wait of each instruction onto preceding
    same-engine EventSemaphore instructions (engines execute in order, so
    this is semantically identical)."""
    m = json.loads(raw)
    ctr = 0
    for fn in m.get("functions", []):
        for blk in fn.get("blocks", []):
            out = []
            for inst in blk.get("instructions", []):
                si = inst.get("sync_info")
                ow = (si or {}).get("on_wait") or []
                if len(ow) > 1:
                    for w in ow[:-1]:
                        ctr += 1
                        out.append({
                            "debug": inst.get("debug", 0),
                            "engine": inst["engine"],
                            "ins": [],
                            "outs": [],
                            "name": f"SWX-{ctr}",
                            "opcode": "EventSemaphore",
                            "sync_info": {"on_update": [], "on_wait": [w]},
                        })
                    si["on_wait"] = [ow[-1]]
                out.append(inst)
            blk["instructions"] = out
    return json.dumps(m).encode()


def _build_tp_kernel(linear_weight: np.ndarray, weight: np.ndarray) -> np.ndarray:
    """Mirror reference.py's CG kernel construction in float64.
    Returns k[5,5,5,64,64] ([dx,dy,dz,in,out]) with the self-connection
    folded into the center tap."""
    lw = linear_weight.astype(np.float64)
    w8 = weight.astype(np.float64)
    ax = np.arange(-2.0, 3.0)
    gx, gy, gz = np.meshgrid(ax, ax, ax, indexing="ij")
    lattice = np.stack([gx, gy, gz], axis=-1)            # [5,5,5,3]
    rad = np.linalg.norm(lattice, axis=-1)
    values = np.linspace(0.0, 2.5, NB + 2)[1:-1]
    step = 2.5 / (NB + 1)
    diff = (rad[..., None] - values) / step
    den = np.maximum(1.0 - diff * diff, 1e-9)
    emb = np.where(np.abs(diff) < 1.0, 1.14136 * np.exp(2.0 - 1.0 / den), 0.0)
    n = rad[..., None]
    unit = np.where(n > 0, lattice / np.where(n > 0, n, 1.0), 0.0)
    sh = np.concatenate([np.ones((5, 5, 5, 1)), math.sqrt(3.0) * unit], -1)
    L = 125
    w = (emb.reshape(L, NB) @ w8) / float(L)             # [125, 1024]
    W = w.reshape(L, 4, MUL, MUL)
    shf = sh.reshape(L, 4)
    y0, y1 = shf[:, 0], shf[:, 1:4]
    a = 1.0 / math.sqrt(2.0 * MUL)
    eye3 = np.eye(3)
    Rss = a * W[:, 0] * y0[:, None, None]
    Rsv = a * np.einsum("luw,lm->luwm", W[:, 1], y1).reshape(L, MUL, 3 * MUL)
    Rvv = a * np.einsum("luw,l,mn->lumwn", W[:, 2], y0, eye3).reshape(L, 3 * MUL, 3 * MUL)
    Rvs = (a / math.sqrt(3.0)) * np.einsum("luw,lm->lumw", W[:, 3], y1).reshape(L, 3 * MUL, MUL)
    k = np.concatenate(
        [np.concatenate([Rss, Rsv], -1), np.concatenate([Rvs, Rvv], -1)], 1
    ).reshape(5, 5, 5, DIM, DIM)
    # self-connection: irrep-wise linear, folded into center tap
    Wl = lw.reshape(2, MUL, MUL) / math.sqrt(MUL)
    sc = np.zeros((DIM, DIM))
    sc[:MUL, :MUL] = Wl[0]
    for m in range(3):
        idx = MUL + np.arange(MUL) * 3 + m
        sc[np.ix_(idx, idx)] = Wl[1]
    k = k.copy()
    k[2, 2, 2] += sc
    return k


def _pack_weights(k: np.ndarray) -> np.ndarray:
    """[128, 65*64] fp16 per-unit weight blocks (rows 64-127 = paired tap,
    zeros when unpaired)."""
    Wp = np.zeros((128, NU * DIM), np.float64)
    for ui, u in enumerate(UNITS):
        s = ui * DIM
        if u[0] == "z":
            _, dx, dy, zg = u
            Wp[0:64, s:s + DIM] = k[dx, dy, 2 * zg]
            Wp[64:128, s:s + DIM] = k[dx, dy, 2 * zg + 1]
        elif u[0] == "y":
            _, dx, yg, _ = u
            Wp[0:64, s:s + DIM] = k[dx, 2 * yg, 4]
            if 2 * yg + 1 < 5:
                Wp[64:128, s:s + DIM] = k[dx, 2 * yg + 1, 4]
        else:
            _, xg, _, _ = u
            Wp[0:64, s:s + DIM] = k[2 * xg, 4, 4]
            if 2 * xg + 1 < 5:
                Wp[64:128, s:s + DIM] = k[2 * xg + 1, 4, 4]
    return Wp.astype(np.float16)


def _build_slabs(xb: np.ndarray, cx: int):
    """Channel-major zero-padded fp16 slab copies for x-chunk cx of
    batch-slice xb [40,40,40,64]:
      xz [128, SLAB_PAD]: rows 0-63 slab, rows 64-127 slab shifted +1 z-voxel
      xy [128, SLAB_PAD]: rows 0-63 slab, rows 64-127 slab shifted +1 y-row
      xx [128, SLAB_PAD]: rows 0-63 slab, rows 64-127 slab shifted +1 x-plane
    (full-128-partition DMA runs at twice the rate of a 64-partition one)."""
    pad = np.zeros((PX, PY, PZ, DIM), np.float32)
    x0 = cx * 10 - 2
    lo, hi = max(0, x0), min(40, x0 + PX)
    pad[lo - x0:hi - x0, 2:42, 2:42, :] = xb[lo:hi]
    xs = np.zeros((DIM, SLAB_PAD), np.float16)
    xs[:, :SLAB] = np.ascontiguousarray(
        pad.transpose(3, 0, 1, 2)).reshape(DIM, SLAB).astype(np.float16)
    xz = np.zeros((128, SLAB_PAD), np.float16)
    xy = np.zeros((128, SLAB_PAD), np.float16)
    xx = np.zeros((128, SLAB_PAD), np.float16)
    for a in (xz, xy, xx):
        a[:DIM] = xs
    xz[DIM:, :SLAB_PAD - 1] = xs[:, 1:]
    xy[DIM:, :SLAB_PAD - PY] = xs[:, PY:]
    xx[DIM:, :SLAB_PAD - PLANE] = xs[:, PLANE:]
    return xz, xy, xx


def _build_program():
    nc = bass.Bass("TRN2", target_bir_lowering=False, debug=False)
    xz_d = nc.dram_tensor("xz", [128, SLAB_PAD], mybir.dt.float16, kind="ExternalInput")
    xy_d = nc.dram_tensor("xy", [128, SLAB_PAD], mybir.dt.float16, kind="ExternalInput")
    xx_d = nc.dram_tensor("xx", [128, SLAB_PAD], mybir.dt.float16, kind="ExternalInput")
    wt_d = nc.dram_tensor("wt", [128, NU * DIM], mybir.dt.float16, kind="ExternalInput")
    y_d = nc.dram_tensor("y", [128, 16000], mybir.dt.float32, kind="ExternalOutput")

    with tile.TileContext(nc) as tc:
        with ExitStack() as ctx:
            wpool = ctx.enter_context(tc.tile_pool(name="wts", bufs=1))
            ppool = ctx.enter_context(tc.tile_pool(name="planes", bufs=1))
            spool = ctx.enter_context(tc.tile_pool(name="stage", bufs=4))
            qpool = ctx.enter_context(tc.tile_pool(name="psum", bufs=2, space="PSUM"))

            # plane tiles first, DMAs issued below in deadline order
            wt_sb = wpool.tile([128, NU * DIM], mybir.dt.float16)
            zplanes = [ppool.tile([128, PLANE], mybir.dt.float16, name=f"pz{i}", tag=f"pz{i}")
                       for i in range(PX)]
            yplanes = [ppool.tile([128, PLANE], mybir.dt.float16, name=f"py{i}", tag=f"py{i}")
                       for i in range(PX)]
            xplanes = [ppool.tile([128, PLANE], mybir.dt.float16, name=f"px{i}", tag=f"px{i}")
                       for i in range(PX)]

            def wt_dma(lo, hi):
                nc.scalar.dma_start(wt_sb[:, lo * DIM:hi * DIM],
                                    wt_d.ap()[:, lo * DIM:hi * DIM])

            def p_dma(eng, tiles, dram, i, c0=0, c1=PLANE):
                o = i * PLANE
                eng.dma_start(tiles[i][:, c0:c1], dram.ap()[:, o + c0:o + c1])

            # Issue order = HW service order per ring; the two rings run in
            # parallel.  The first matmul gates on wt units 0-3 + z-plane 0,
            # so z0 is split across BOTH rings (2x bandwidth) and the first
            # weight chunk is tiny.  Later planes are ordered by the time
            # the unit stream consumes them (x0/x2/x4 before plane 0's three
            # x-pair units, etc.).
            # ACT ring: z0 back half, wt head, remaining wt, y-planes.
            p_dma(nc.scalar, zplanes, xz_d, 0, PLANE // 2, PLANE)
            wt_dma(0, 4)
            wt_dma(4, 20)
            p_dma(nc.scalar, yplanes, xy_d, 0)
            wt_dma(20, 44)
            p_dma(nc.scalar, yplanes, xy_d, 1)
            wt_dma(44, 68)
            for i in range(2, PX):
                p_dma(nc.scalar, yplanes, xy_d, i)
            # SP ring: z0 front half, z1-z4, then x/z interleaved by deadline.
            p_dma(nc.sync, zplanes, xz_d, 0, 0, PLANE // 2)
            for i in (1, 2, 3, 4):
                p_dma(nc.sync, zplanes, xz_d, i)
            for kind, i in (("x", 0), ("x", 2), ("z", 5), ("x", 4), ("z", 6),
                            ("x", 1), ("x", 3), ("x", 5)):
                p_dma(nc.sync, zplanes if kind == "z" else xplanes,
                      xz_d if kind == "z" else xx_d, i)
            for i in range(7, PX):
                p_dma(nc.sync, zplanes, xz_d, i)
            for i in range(6, PX):
                p_dma(nc.sync, xplanes, xx_d, i)

            def plane(kind, i):
                g = {"z": zplanes, "y": yplanes, "x": xplanes}[kind][i]
                return g[:].rearrange("p (y z) -> p y z", y=PY)

            def unit_src(u, px, ty):
                if u[0] == "z":
                    _, dx, dy, zg = u
                    kind, dyy, zo = "z", dy, 2 * zg
                elif u[0] == "y":
                    _, dx, yg, _ = u
                    kind, dyy, zo = "y", 2 * yg, 4
                else:
                    _, xg, _, _ = u
                    dx, kind, dyy, zo = 2 * xg, "x", 4, 4
                yb = ty * 10 + dyy
                return plane(kind, px + dx)[:, yb:yb + 10, zo:zo + 40]

            nu = len(U63)
            for px in range(10):
                stage = spool.tile([128, 1600], mybir.dt.float32, name="stage", tag="stage")
                last = px == 9
                if not last:
                    # ty-innermost: 4 matmuls share one weight load
                    pss = [qpool.tile([128, 400], mybir.dt.float32, name=f"ps{t}", tag=f"ps{t}")
                           for t in range(4)]
                    first = [True, True]
                    for pos, ui in enumerate(U63):
                        grp = pos % 2
                        for ty in range(4):
                            nc.tensor.matmul(
                                pss[ty][grp * 64:(grp + 1) * 64, :],
                                wt_sb[:, ui * DIM:(ui + 1) * DIM],
                                unit_src(UNITS[ui], px, ty),
                                start=first[grp],
                                stop=(pos >= nu - 2),
                                tile_position=(0, grp * 64),
                            )
                        first[grp] = False
                    for ty in range(4):
                        nc.vector.tensor_copy(stage[:, ty * 400:(ty + 1) * 400], pss[ty][:])
                        nc.scalar.dma_start(
                            y_d.ap()[:, px * 1600 + ty * 400:px * 1600 + (ty + 1) * 400],
                            stage[:, ty * 400:(ty + 1) * 400])
                else:
                    # last plane: per-ty accumulation so only the final
                    # quarter's evacuation is exposed at the kernel tail;
                    # the final quarter itself runs as two N=200 half-chains
                    # so its first half drains while the second computes
                    for ty in range(4):
                        halves = ((0, 400),) if ty < 3 else ((0, 280), (280, 400))
                        for h0, h1 in halves:
                            # the 120-col final piece gets its own PSUM tile
                            # (tag ps1 is free again) so its first matmuls
                            # don't wait on the 280-col piece's evacuation
                            tag = f"ps{ty}" if h0 == 0 else "ps1"
                            ps = qpool.tile([128, h1 - h0], mybir.dt.float32,
                                            name=f"ps{ty}_{h0}", tag=tag)
                            yr0, nrow = h0 // 40, (h1 - h0) // 40
                            first = [True, True]
                            for pos, ui in enumerate(U63):
                                grp = pos % 2
                                src = unit_src(UNITS[ui], px, ty)
                                nc.tensor.matmul(
                                    ps[grp * 64:(grp + 1) * 64, :],
                                    wt_sb[:, ui * DIM:(ui + 1) * DIM],
                                    src[:, yr0:yr0 + nrow, :],
                                    start=first[grp],
                                    stop=(pos >= nu - 2),
                                    tile_position=(0, grp * 64),
                                )
                                first[grp] = False
                            nc.vector.tensor_copy(stage[:, ty * 400 + h0:ty * 400 + h1],
                                                  ps[:])
                            # final piece goes out on the idle SP ring so the
                            # ACT queue's end-of-program drain overlaps it
                            ring = nc.sync if h0 > 0 else nc.scalar
                            ring.dma_start(
                                y_d.ap()[:, px * 1600 + ty * 400 + h0:px * 1600 + ty * 400 + h1],
                                stage[:, ty * 400 + h0:ty * 400 + h1])

    orig = nc.to_json_bytes
    nc.to_json_bytes = functools.wraps(orig)(lambda: _split_sync_waits_json(orig()))
    return nc


def kernel(x, linear_weight, weight, _trace=False):
    x = np.asarray(x, np.float32)
    k = _build_tp_kernel(np.asarray(linear_weight), np.asarray(weight))
    wt = _pack_weights(k)

    in_maps = []
    for core in range(8):
        b, cx = divmod(core, 4)
        xz, xy, xx = _build_slabs(x[b], cx)
        in_maps.append({"xz": xz, "xy": xy, "xx": xx, "wt": wt})

    nc = _build_program()
    res = run_bass_kernel_spmd(nc, in_maps, core_ids=list(range(8)), trace=_trace)

    y = np.empty((2, 40, 40, 40, DIM), np.float32)
    for core in range(8):
        b, cx = divmod(core, 4)
        yc = res.results[core]["y"]
        s = (yc[:64] + yc[64:]).reshape(DIM, 10, 4, 10, 40)
        y[b, cx * 10:(cx + 1) * 10] = s.transpose(1, 2, 3, 4, 0).reshape(10, 40, 40, DIM)
    if _trace:
        kernel.last_results = res
    return y



# revision 12
# speedup vs baseline: 1.0036x; 1.0036x over previous
"""Trainium2 Bass kernel for nn_Convolution_77111842832763.

3D conv 5x5x5 SAME, 64->64 channels, input [2,40,40,40,64] fp32, plus an
irrep-wise linear self-connection (folded into the conv's center tap).

Strategy (8 NeuronCores, data-parallel):
  - Shard: core = batch(2) x x-chunk(4); each core computes a [10,40,40,64]
    output slab from a zero-padded [14,44,44,64] input slab (halo 2).
  - Host builds the 5^3 x 64 x 64 tensor-product kernel exactly (float64),
    folds the self-connection into the center tap, and packs per-tap weight
    blocks; weights/slabs are cast to fp16 on host (device matmuls run fp16
    with fp32 PSUM accumulation; measured end-to-end rel-err ~2.9e-4).
  - Device: TWO channel-major slab copies in SBUF [128, 14*44*44]:
      slab_z: partitions 0-63 = slab, 64-127 = slab shifted +1 z-voxel
      slab_y: partitions 0-63 = slab, 64-127 = slab shifted +1 y-row
    K=128 packing: the 125 taps become 63 matmul units per output tile:
      50 z-pair units  (dx,dy, dz in {(0,1),(2,3)})        -> slab_z
      10 y-pair units  (dx, dz=4 slice, dy in {(0,1),(2,3)}) -> slab_y
       3 x-pair units  (dy=4,dz=4 column, dx in {(0,1),(2,3),(4,zero)}) -> slab_x
  - Units alternate between PE column groups 0-63/64-127 (2x column
    tiling) accumulating into psum[0:64]/psum[64:128]; the two partial
    sums are DMA'd out separately and added on host.
  - Output tile = one x-plane quarter: 10 y-rows x 40 z = 400 voxels
    (moving free dim 400, one PSUM bank).  40 tiles per core.
  - DMA: z/x-copy planes on the SP HWDGE ring, weights/y-planes/outputs
    on the ACT ring (parallel rings), each ring ordered by consumption
    deadline; z-plane 0 is split across both rings so the first matmul's
    gate lands ~2x sooner.  A JSON post-pass splits multi-wait
    instructions (this walrus build allows one sync wait per instruction).
"""

import functools
import json
import math
from contextlib import ExitStack

import numpy as np

import concourse.bass as bass
import concourse.mybir as mybir
import concourse.tile as tile
from concourse.bass_utils import run_bass_kernel_spmd

MUL = 16
DIM = 64
NB = 8
PX, PY, PZ = 14, 44, 44          # padded slab dims
PLANE = PY * PZ                   # 1936
SLAB = PX * PLANE                 # 27104
SLAB_PAD = SLAB + 48              # tail zeros so the +1z and +44y views stay in-bounds
# unit list: ("z", dx, dy, zg) -> taps (dx,dy,2*zg)+(dx,dy,2*zg+1) via slab_z
#            ("y", dx, yg)     -> taps (dx,2*yg,4)+(dx,2*yg+1,4)   via slab_y
#            ("x", xg)         -> taps (2*xg,4,4)+(2*xg+1,4,4)     via slab_x
# The weight table holds all 68 unit blocks; every output plane runs the
# 63-unit scheme (the input DMA order streams the x-copy planes early
# enough that even plane 0's three x-pair units arrive in time).
UNITS = [("z", dx, dy, zg) for dx in range(5) for dy in range(5) for zg in range(2)]
UNITS += [("y", dx, yg, 0) for dx in range(5) for yg in range(3)]
UNITS += [("x", xg, 0, 0) for xg in range(3)]
NU = len(UNITS)                   # 68 weight blocks
U63 = list(range(50)) + [50 + 3 * dx + yg for dx in range(5) for yg in range(2)] \
    + [65, 66, 67]                                         # z50 + y10 + x3


def _split_sync_waits_json(raw: bytes) -> bytes:
    """Hoist all but the last sync wait of each instruction onto preceding
    same-engine EventSemaphore instructions (engines execute in order, so
    this is semantically identical)."""
    m = json.loads(raw)
    ctr = 0
    for fn in m.get("functions", []):
        for blk in fn.get("blocks", []):
            out = []
            for inst in blk.get("instructions", []):
                si = inst.get("sync_info")
                ow = (si or {}).get("on_wait") or []
                if len(ow) > 1:
                    for w in ow[:-1]:
                        ctr += 1
                        out.append({
                            "debug": inst.get("debug", 0),
                            "engine": inst["engine"],
                            "ins": [],
                            "outs": [],
                            "name": f"SWX-{ctr}",
                            "opcode": "EventSemaphore",
                            "sync_info": {"on_update": [], "on_wait": [w]},
                        })
                    si["on_wait"] = [ow[-1]]
                out.append(inst)
            blk["instructions"] = out
    return json.dumps(m).encode()


def _build_tp_kernel(linear_weight: np.ndarray, weight: np.ndarray) -> np.ndarray:
    """Mirror reference.py's CG kernel construction in float64.
    Returns k[5,5,5,64,64] ([dx,dy,dz,in,out]) with the self-connection
    folded into the center tap."""
    lw = linear_weight.astype(np.float64)
    w8 = weight.astype(np.float64)
    ax = np.arange(-2.0, 3.0)
    gx, gy, gz = np.meshgrid(ax, ax, ax, indexing="ij")
    lattice = np.stack([gx, gy, gz], axis=-1)            # [5,5,5,3]
    rad = np.linalg.norm(lattice, axis=-1)
    values = np.linspace(0.0, 2.5, NB + 2)[1:-1]
    step = 2.5 / (NB + 1)
    diff = (rad[..., None] - values) / step
    den = np.maximum(1.0 - diff * diff, 1e-9)
    emb = np.where(np.abs(diff) < 1.0, 1.14136 * np.exp(2.0 - 1.0 / den), 0.0)
    n = rad[..., None]
    unit = np.where(n > 0, lattice / np.where(n > 0, n, 1.0), 0.0)
    sh = np.concatenate([np.ones((5, 5, 5, 1)), math.sqrt(3.0) * unit], -1)
    L = 125
    w = (emb.reshape(L, NB) @ w8) / float(L)             # [125, 1024]
    W = w.reshape(L, 4, MUL, MUL)
    shf = sh.reshape(L, 4)
    y0, y1 = shf[:, 0], shf[:, 1:4]
    a = 1.0 / math.sqrt(2.0 * MUL)
    eye3 = np.eye(3)
    Rss = a * W[:, 0] * y0[:, None, None]
    Rsv = a * np.einsum("luw,lm->luwm", W[:, 1], y1).reshape(L, MUL, 3 * MUL)
    Rvv = a * np.einsum("luw,l,mn->lumwn", W[:, 2], y0, eye3).reshape(L, 3 * MUL, 3 * MUL)
    Rvs = (a / math.sqrt(3.0)) * np.einsum("luw,lm->lumw", W[:, 3], y1).reshape(L, 3 * MUL, MUL)
    k = np.concatenate(
        [np.concatenate([Rss, Rsv], -1), np.concatenate([Rvs, Rvv], -1)], 1
    ).reshape(5, 5, 5, DIM, DIM)
    # self-connection: irrep-wise linear, folded into center tap
    Wl = lw.reshape(2, MUL, MUL) / math.sqrt(MUL)
    sc = np.zeros((DIM, DIM))
    sc[:MUL, :MUL] = Wl[0]
    for m in range(3):
        idx = MUL + np.arange(MUL) * 3 + m
        sc[np.ix_(idx, idx)] = Wl[1]
    k = k.copy()
    k[2, 2, 2] += sc
    return k


def _pack_weights(k: np.ndarray) -> np.ndarray:
    """[128, 65*64] fp16 per-unit weight blocks (rows 64-127 = paired tap,
    zeros when unpaired)."""
    Wp = np.zeros((128, NU * DIM), np.float64)
    for ui, u in enumerate(UNITS):
        s = ui * DIM
        if u[0] == "z":
            _, dx, dy, zg = u
            Wp[0:64, s:s + DIM] = k[dx, dy, 2 * zg]
            Wp[64:128, s:s + DIM] = k[dx, dy, 2 * zg + 1]
        elif u[0] == "y":
            _, dx, yg, _ = u
            Wp[0:64, s:s + DIM] = k[dx, 2 * yg, 4]
            if 2 * yg + 1 < 5:
                Wp[64:128, s:s + DIM] = k[dx, 2 * yg + 1, 4]
        else:
            _, xg, _, _ = u
            Wp[0:64, s:s + DIM] = k[2 * xg, 4, 4]
            if 2 * xg + 1 < 5:
                Wp[64:128, s:s + DIM] = k[2 * xg + 1, 4, 4]
    return Wp.astype(np.float16)


def _build_slabs(xb: np.ndarray, cx: int):
    """Channel-major zero-padded fp16 slab copies for x-chunk cx of
    batch-slice xb [40,40,40,64]:
      xz [128, SLAB_PAD]: rows 0-63 slab, rows 64-127 slab shifted +1 z-voxel
      xy [128, SLAB_PAD]: rows 0-63 slab, rows 64-127 slab shifted +1 y-row
      xx [128, SLAB_PAD]: rows 0-63 slab, rows 64-127 slab shifted +1 x-plane
    (full-128-partition DMA runs at twice the rate of a 64-partition one)."""
    pad = np.zeros((PX, PY, PZ, DIM), np.float32)
    x0 = cx * 10 - 2
    lo, hi = max(0, x0), min(40, x0 + PX)
    pad[lo - x0:hi - x0, 2:42, 2:42, :] = xb[lo:hi]
    xs = np.zeros((DIM, SLAB_PAD), np.float16)
    xs[:, :SLAB] = np.ascontiguousarray(
        pad.transpose(3, 0, 1, 2)).reshape(DIM, SLAB).astype(np.float16)
    xz = np.zeros((128, SLAB_PAD), np.float16)
    xy = np.zeros((128, SLAB_PAD), np.float16)
    xx = np.zeros((128, SLAB_PAD), np.float16)
    for a in (xz, xy, xx):
        a[:DIM] = xs
    xz[DIM:, :SLAB_PAD - 1] = xs[:, 1:]
    xy[DIM:, :SLAB_PAD - PY] = xs[:, PY:]
    xx[DIM:, :SLAB_PAD - PLANE] = xs[:, PLANE:]
    return xz, xy, xx


def _build_program():
    nc = bass.Bass("TRN2", target_bir_lowering=False, debug=False)
    xz_d = nc.dram_tensor("xz", [128, SLAB_PAD], mybir.dt.float16, kind="ExternalInput")
    xy_d = nc.dram_tensor("xy", [128, SLAB_PAD], mybir.dt.float16, kind="ExternalInput")
    xx_d = nc.dram_tensor("xx", [128, SLAB_PAD], mybir.dt.float16, kind="ExternalInput")
    wt_d = nc.dram_tensor("wt", [128, NU * DIM], mybir.dt.float16, kind="ExternalInput")
    y_d = nc.dram_tensor("y", [128, 16000], mybir.dt.float32, kind="ExternalOutput")

    with tile.TileContext(nc) as tc:
        with ExitStack() as ctx:
            wpool = ctx.enter_context(tc.tile_pool(name="wts", bufs=1))
            ppool = ctx.enter_context(tc.tile_pool(name="planes", bufs=1))
            spool = ctx.enter_context(tc.tile_pool(name="stage", bufs=4))
            qpool = ctx.enter_context(tc.tile_pool(name="psum", bufs=2, space="PSUM"))

            # plane tiles first, DMAs issued below in deadline order
            wt_sb = wpool.tile([128, NU * DIM], mybir.dt.float16)
            zplanes = [ppool.tile([128, PLANE], mybir.dt.float16, name=f"pz{i}", tag=f"pz{i}")
                       for i in range(PX)]
            yplanes = [ppool.tile([128, PLANE], mybir.dt.float16, name=f"py{i}", tag=f"py{i}")
                       for i in range(PX)]
            xplanes = [ppool.tile([128, PLANE], mybir.dt.float16, name=f"px{i}", tag=f"px{i}")
                       for i in range(PX)]

            def wt_dma(lo, hi):
                nc.scalar.dma_start(wt_sb[:, lo * DIM:hi * DIM],
                                    wt_d.ap()[:, lo * DIM:hi * DIM])

            def p_dma(eng, tiles, dram, i, c0=0, c1=PLANE):
                o = i * PLANE
                eng.dma_start(tiles[i][:, c0:c1], dram.ap()[:, o + c0:o + c1])

            # Issue order = HW service order per ring; the two rings run in
            # parallel.  The first matmul gates on wt units 0-3 + z-plane 0,
            # so z0 is split across BOTH rings (2x bandwidth) and the first
            # weight chunk is tiny.  Later planes are ordered by the time
            # the unit stream consumes them (x0/x2/x4 before plane 0's three
            # x-pair units, etc.).
            # ACT ring: z0 back half, wt head, remaining wt, y-planes.
            p_dma(nc.scalar, zplanes, xz_d, 0, PLANE // 2, PLANE)
            wt_dma(0, 4)
            wt_dma(4, 20)
            p_dma(nc.scalar, yplanes, xy_d, 0)
            wt_dma(20, 44)
            p_dma(nc.scalar, yplanes, xy_d, 1)
            wt_dma(44, 68)
            for i in range(2, PX):
                p_dma(nc.scalar, yplanes, xy_d, i)
            # SP ring: z0 front half, z1-z4, then x/z interleaved by deadline.
            p_dma(nc.sync, zplanes, xz_d, 0, 0, PLANE // 2)
            for i in (1, 2, 3, 4):
                p_dma(nc.sync, zplanes, xz_d, i)
            for kind, i in (("x", 0), ("x", 2), ("z", 5), ("x", 4), ("z", 6),
                            ("x", 1), ("x", 3), ("x", 5)):
                p_dma(nc.sync, zplanes if kind == "z" else xplanes,
                      xz_d if kind == "z" else xx_d, i)
            for i in range(7, PX):
                p_dma(nc.sync, zplanes, xz_d, i)
            for i in range(6, PX):
                p_dma(nc.sync, xplanes, xx_d, i)

            def plane(kind, i):
                g = {"z": zplanes, "y": yplanes, "x": xplanes}[kind][i]
                return g[:].rearrange("p (y z) -> p y z", y=PY)

            def unit_src(u, px, ty):
                if u[0] == "z":
                    _, dx, dy, zg = u
                    kind, dyy, zo = "z", dy, 2 * zg
                elif u[0] == "y":
                    _, dx, yg, _ = u
                    kind, dyy, zo = "y", 2 * yg, 4
                else:
                    _, xg, _, _ = u
                    dx, kind, dyy, zo = 2 * xg, "x", 4, 4
                yb = ty * 10 + dyy
                return plane(kind, px + dx)[:, yb:yb + 10, zo:zo + 40]

            nu = len(U63)
            for px in range(10):
                stage = spool.tile([128, 1600], mybir.dt.float32, name="stage", tag="stage")
                last = px == 9
                if not last:
                    # ty-innermost: 4 matmuls share one weight load
                    pss = [qpool.tile([128, 400], mybir.dt.float32, name=f"ps{t}", tag=f"ps{t}")
                           for t in range(4)]
                    first = [True, True]
                    for pos, ui in enumerate(U63):
                        grp = pos % 2
                        for ty in range(4):
                            nc.tensor.matmul(
                                pss[ty][grp * 64:(grp + 1) * 64, :],
                                wt_sb[:, ui * DIM:(ui + 1) * DIM],
                                unit_src(UNITS[ui], px, ty),
                                start=first[grp],
                                stop=(pos >= nu - 2),
                                tile_position=(0, grp * 64),
                            )
                        first[grp] = False
                    for ty in range(4):
                        nc.vector.tensor_copy(stage[:, ty * 400:(ty + 1) * 400], pss[ty][:])
                        nc.scalar.dma_start(
                            y_d.ap()[:, px * 1600 + ty * 400:px * 1600 + (ty + 1) * 400],
                            stage[:, ty * 400:(ty + 1) * 400])
                else:
                    # last plane: per-ty accumulation so only the final
                    # quarter's evacuation is exposed at the kernel tail;
                    # the final quarter itself runs as 280+120-col chains
                    # so most of it drains while the 120-col piece computes
                    for ty in range(4):
                        halves = ((0, 400),) if ty < 3 else ((0, 280), (280, 400))
                        for h0, h1 in halves:
                            # the 120-col final piece gets its own PSUM tile
                            # (tag ps1 is free again) so its first matmuls
                            # don't wait on the 280-col piece's evacuation
                            tag = f"ps{ty}" if h0 == 0 else "ps1"
                            ps = qpool.tile([128, h1 - h0], mybir.dt.float32,
                                            name=f"ps{ty}_{h0}", tag=tag)
                            yr0, nrow = h0 // 40, (h1 - h0) // 40
                            first = [True, True]
                            for pos, ui in enumerate(U63):
                                grp = pos % 2
                                src = unit_src(UNITS[ui], px, ty)
                                nc.tensor.matmul(
                                    ps[grp * 64:(grp + 1) * 64, :],
                                    wt_sb[:, ui * DIM:(ui + 1) * DIM],
                                    src[:, yr0:yr0 + nrow, :],
                                    start=first[grp],
                                    stop=(pos >= nu - 2),
                                    tile_position=(0, grp * 64),
                                )
                                first[grp] = False
                            nc.vector.tensor_copy(stage[:, ty * 400 + h0:ty * 400 + h1],
                                                  ps[:])
                            # final piece goes out on the idle SP ring so the
                            # ACT queue's end-of-program drain overlaps it
                            ring = nc.sync if h0 > 0 else nc.scalar
                            ring.dma_start(
                                y_d.ap()[:, px * 1600 + ty * 400 + h0:px * 1600 + ty * 400 + h1],
                                stage[:, ty * 400 + h0:ty * 400 + h1])

    orig = nc.to_json_bytes
    nc.to_json_bytes = functools.wraps(orig)(lambda: _split_sync_waits_json(orig()))
    return nc


def kernel(x, linear_weight, weight, _trace=False):
    x = np.asarray(x, np.float32)
    k = _build_tp_kernel(np.asarray(linear_weight), np.asarray(weight))
    wt = _pack_weights(k)

    in_maps = []
    for core in range(8):
        b, cx = divmod(core, 4)
        xz, xy, xx = _build_slabs(x[b], cx)
        in_maps.append({"xz": xz, "xy": xy, "xx": xx, "wt": wt})

    nc = _build_program()
    res = run_bass_kernel_spmd(nc, in_maps, core_ids=list(range(8)), trace=_trace)

    y = np.empty((2, 40, 40, 40, DIM), np.float32)
    for core in range(8):
        b, cx = divmod(core, 4)
        yc = res.results[core]["y"]
        s = (yc[:64] + yc[64:]).reshape(DIM, 10, 4, 10, 40)
        y[b, cx * 10:(cx + 1) * 10] = s.transpose(1, 2, 3, 4, 0).reshape(10, 40, 40, DIM)
    if _trace:
        kernel.last_results = res
    return y



# revision 13
# speedup vs baseline: 1.0052x; 1.0016x over previous
"""Trainium2 Bass kernel for nn_Convolution_77111842832763.

3D conv 5x5x5 SAME, 64->64 channels, input [2,40,40,40,64] fp32, plus an
irrep-wise linear self-connection (folded into the conv's center tap).

Strategy (8 NeuronCores, data-parallel):
  - Shard: core = batch(2) x x-chunk(4); each core computes a [10,40,40,64]
    output slab from a zero-padded [14,44,44,64] input slab (halo 2).
  - Host builds the 5^3 x 64 x 64 tensor-product kernel exactly (float64),
    folds the self-connection into the center tap, and packs per-tap weight
    blocks; weights/slabs are cast to fp16 on host (device matmuls run fp16
    with fp32 PSUM accumulation; measured end-to-end rel-err ~2.9e-4).
  - Device: TWO channel-major slab copies in SBUF [128, 14*44*44]:
      slab_z: partitions 0-63 = slab, 64-127 = slab shifted +1 z-voxel
      slab_y: partitions 0-63 = slab, 64-127 = slab shifted +1 y-row
    K=128 packing: the 125 taps become 63 matmul units per output tile:
      50 z-pair units  (dx,dy, dz in {(0,1),(2,3)})        -> slab_z
      10 y-pair units  (dx, dz=4 slice, dy in {(0,1),(2,3)}) -> slab_y
       3 x-pair units  (dy=4,dz=4 column, dx in {(0,1),(2,3),(4,zero)}) -> slab_x
  - Units alternate between PE column groups 0-63/64-127 (2x column
    tiling) accumulating into psum[0:64]/psum[64:128]; the two partial
    sums are DMA'd out separately and added on host.
  - Output tile = one x-plane quarter: 10 y-rows x 40 z = 400 voxels
    (moving free dim 400, one PSUM bank).  40 tiles per core.
  - DMA: z/x-copy planes on the SP HWDGE ring, weights/y-planes/outputs
    on the ACT ring (parallel rings), each ring ordered by consumption
    deadline; z-plane 0 is split across both rings so the first matmul's
    gate lands ~2x sooner.  A JSON post-pass splits multi-wait
    instructions (this walrus build allows one sync wait per instruction).
"""

import functools
import json
import math
from contextlib import ExitStack

import numpy as np

import concourse.bass as bass
import concourse.mybir as mybir
import concourse.tile as tile
from concourse.bass_utils import run_bass_kernel_spmd

MUL = 16
DIM = 64
NB = 8
PX, PY, PZ = 14, 44, 44          # padded slab dims
PLANE = PY * PZ                   # 1936
SLAB = PX * PLANE                 # 27104
SLAB_PAD = SLAB + 48              # tail zeros so the +1z and +44y views stay in-bounds
# unit list: ("z", dx, dy, zg) -> taps (dx,dy,2*zg)+(dx,dy,2*zg+1) via slab_z
#            ("y", dx, yg)     -> taps (dx,2*yg,4)+(dx,2*yg+1,4)   via slab_y
#            ("x", xg)         -> taps (2*xg,4,4)+(2*xg+1,4,4)     via slab_x
# The weight table holds all 68 unit blocks; every output plane runs the
# 63-unit scheme (the input DMA order streams the x-copy planes early
# enough that even plane 0's three x-pair units arrive in time).
UNITS = [("z", dx, dy, zg) for dx in range(5) for dy in range(5) for zg in range(2)]
UNITS += [("y", dx, yg, 0) for dx in range(5) for yg in range(3)]
UNITS += [("x", xg, 0, 0) for xg in range(3)]
NU = len(UNITS)                   # 68 weight blocks
U63 = list(range(50)) + [50 + 3 * dx + yg for dx in range(5) for yg in range(2)] \
    + [65, 66, 67]                                         # z50 + y10 + x3


def _split_sync_waits_json(raw: bytes) -> bytes:
    """Hoist all but the last sync wait of each instruction onto preceding
    same-engine EventSemaphore instructions (engines execute in order, so
    this is semantically identical)."""
    m = json.loads(raw)
    ctr = 0
    for fn in m.get("functions", []):
        for blk in fn.get("blocks", []):
            out = []
            for inst in blk.get("instructions", []):
                si = inst.get("sync_info")
                ow = (si or {}).get("on_wait") or []
                if len(ow) > 1:
                    for w in ow[:-1]:
                        ctr += 1
                        out.append({
                            "debug": inst.get("debug", 0),
                            "engine": inst["engine"],
                            "ins": [],
                            "outs": [],
                            "name": f"SWX-{ctr}",
                            "opcode": "EventSemaphore",
                            "sync_info": {"on_update": [], "on_wait": [w]},
                        })
                    si["on_wait"] = [ow[-1]]
                out.append(inst)
            blk["instructions"] = out
    return json.dumps(m).encode()


def _build_tp_kernel(linear_weight: np.ndarray, weight: np.ndarray) -> np.ndarray:
    """Mirror reference.py's CG kernel construction in float64.
    Returns k[5,5,5,64,64] ([dx,dy,dz,in,out]) with the self-connection
    folded into the center tap."""
    lw = linear_weight.astype(np.float64)
    w8 = weight.astype(np.float64)
    ax = np.arange(-2.0, 3.0)
    gx, gy, gz = np.meshgrid(ax, ax, ax, indexing="ij")
    lattice = np.stack([gx, gy, gz], axis=-1)            # [5,5,5,3]
    rad = np.linalg.norm(lattice, axis=-1)
    values = np.linspace(0.0, 2.5, NB + 2)[1:-1]
    step = 2.5 / (NB + 1)
    diff = (rad[..., None] - values) / step
    den = np.maximum(1.0 - diff * diff, 1e-9)
    emb = np.where(np.abs(diff) < 1.0, 1.14136 * np.exp(2.0 - 1.0 / den), 0.0)
    n = rad[..., None]
    unit = np.where(n > 0, lattice / np.where(n > 0, n, 1.0), 0.0)
    sh = np.concatenate([np.ones((5, 5, 5, 1)), math.sqrt(3.0) * unit], -1)
    L = 125
    w = (emb.reshape(L, NB) @ w8) / float(L)             # [125, 1024]
    W = w.reshape(L, 4, MUL, MUL)
    shf = sh.reshape(L, 4)
    y0, y1 = shf[:, 0], shf[:, 1:4]
    a = 1.0 / math.sqrt(2.0 * MUL)
    eye3 = np.eye(3)
    Rss = a * W[:, 0] * y0[:, None, None]
    Rsv = a * np.einsum("luw,lm->luwm", W[:, 1], y1).reshape(L, MUL, 3 * MUL)
    Rvv = a * np.einsum("luw,l,mn->lumwn", W[:, 2], y0, eye3).reshape(L, 3 * MUL, 3 * MUL)
    Rvs = (a / math.sqrt(3.0)) * np.einsum("luw,lm->lumw", W[:, 3], y1).reshape(L, 3 * MUL, MUL)
    k = np.concatenate(
        [np.concatenate([Rss, Rsv], -1), np.concatenate([Rvs, Rvv], -1)], 1
    ).reshape(5, 5, 5, DIM, DIM)
    # self-connection: irrep-wise linear, folded into center tap
    Wl = lw.reshape(2, MUL, MUL) / math.sqrt(MUL)
    sc = np.zeros((DIM, DIM))
    sc[:MUL, :MUL] = Wl[0]
    for m in range(3):
        idx = MUL + np.arange(MUL) * 3 + m
        sc[np.ix_(idx, idx)] = Wl[1]
    k = k.copy()
    k[2, 2, 2] += sc
    return k


def _pack_weights(k: np.ndarray) -> np.ndarray:
    """[128, 65*64] fp16 per-unit weight blocks (rows 64-127 = paired tap,
    zeros when unpaired)."""
    Wp = np.zeros((128, NU * DIM), np.float64)
    for ui, u in enumerate(UNITS):
        s = ui * DIM
        if u[0] == "z":
            _, dx, dy, zg = u
            Wp[0:64, s:s + DIM] = k[dx, dy, 2 * zg]
            Wp[64:128, s:s + DIM] = k[dx, dy, 2 * zg + 1]
        elif u[0] == "y":
            _, dx, yg, _ = u
            Wp[0:64, s:s + DIM] = k[dx, 2 * yg, 4]
            if 2 * yg + 1 < 5:
                Wp[64:128, s:s + DIM] = k[dx, 2 * yg + 1, 4]
        else:
            _, xg, _, _ = u
            Wp[0:64, s:s + DIM] = k[2 * xg, 4, 4]
            if 2 * xg + 1 < 5:
                Wp[64:128, s:s + DIM] = k[2 * xg + 1, 4, 4]
    return Wp.astype(np.float16)


def _build_slabs(xb: np.ndarray, cx: int):
    """Channel-major zero-padded fp16 slab copies for x-chunk cx of
    batch-slice xb [40,40,40,64]:
      xz [128, SLAB_PAD]: rows 0-63 slab, rows 64-127 slab shifted +1 z-voxel
      xy [128, SLAB_PAD]: rows 0-63 slab, rows 64-127 slab shifted +1 y-row
      xx [128, SLAB_PAD]: rows 0-63 slab, rows 64-127 slab shifted +1 x-plane
    (full-128-partition DMA runs at twice the rate of a 64-partition one)."""
    pad = np.zeros((PX, PY, PZ, DIM), np.float32)
    x0 = cx * 10 - 2
    lo, hi = max(0, x0), min(40, x0 + PX)
    pad[lo - x0:hi - x0, 2:42, 2:42, :] = xb[lo:hi]
    xs = np.zeros((DIM, SLAB_PAD), np.float16)
    xs[:, :SLAB] = np.ascontiguousarray(
        pad.transpose(3, 0, 1, 2)).reshape(DIM, SLAB).astype(np.float16)
    xz = np.zeros((128, SLAB_PAD), np.float16)
    xy = np.zeros((128, SLAB_PAD), np.float16)
    xx = np.zeros((128, SLAB_PAD), np.float16)
    for a in (xz, xy, xx):
        a[:DIM] = xs
    xz[DIM:, :SLAB_PAD - 1] = xs[:, 1:]
    xy[DIM:, :SLAB_PAD - PY] = xs[:, PY:]
    xx[DIM:, :SLAB_PAD - PLANE] = xs[:, PLANE:]
    return xz, xy, xx


def _build_program():
    nc = bass.Bass("TRN2", target_bir_lowering=False, debug=False)
    xz_d = nc.dram_tensor("xz", [128, SLAB_PAD], mybir.dt.float16, kind="ExternalInput")
    xy_d = nc.dram_tensor("xy", [128, SLAB_PAD], mybir.dt.float16, kind="ExternalInput")
    xx_d = nc.dram_tensor("xx", [128, SLAB_PAD], mybir.dt.float16, kind="ExternalInput")
    wt_d = nc.dram_tensor("wt", [128, NU * DIM], mybir.dt.float16, kind="ExternalInput")
    y_d = nc.dram_tensor("y", [128, 16000], mybir.dt.float32, kind="ExternalOutput")

    with tile.TileContext(nc) as tc:
        with ExitStack() as ctx:
            wpool = ctx.enter_context(tc.tile_pool(name="wts", bufs=1))
            ppool = ctx.enter_context(tc.tile_pool(name="planes", bufs=1))
            spool = ctx.enter_context(tc.tile_pool(name="stage", bufs=4))
            qpool = ctx.enter_context(tc.tile_pool(name="psum", bufs=2, space="PSUM"))

            # plane tiles first, DMAs issued below in deadline order
            wt_sb = wpool.tile([128, NU * DIM], mybir.dt.float16)
            zplanes = [ppool.tile([128, PLANE], mybir.dt.float16, name=f"pz{i}", tag=f"pz{i}")
                       for i in range(PX)]
            yplanes = [ppool.tile([128, PLANE], mybir.dt.float16, name=f"py{i}", tag=f"py{i}")
                       for i in range(PX)]
            xplanes = [ppool.tile([128, PLANE], mybir.dt.float16, name=f"px{i}", tag=f"px{i}")
                       for i in range(PX)]

            def wt_dma(lo, hi):
                nc.scalar.dma_start(wt_sb[:, lo * DIM:hi * DIM],
                                    wt_d.ap()[:, lo * DIM:hi * DIM])

            def p_dma(eng, tiles, dram, i, c0=0, c1=PLANE):
                o = i * PLANE
                eng.dma_start(tiles[i][:, c0:c1], dram.ap()[:, o + c0:o + c1])

            # Issue order = HW service order per ring; the two rings run in
            # parallel.  The first matmul gates on wt units 0-3 + z-plane 0,
            # so z0 is split across BOTH rings (2x bandwidth) and the first
            # weight chunk is tiny.  Later planes are ordered by the time
            # the unit stream consumes them (x0/x2/x4 before plane 0's three
            # x-pair units, etc.).
            # z-plane 0 in three pieces: ty0's rows land first on SP, ty1's
            # on ACT (behind the tiny wt head), ty2/ty3's second on SP —
            # the first matmul gates on just 158KB instead of 248KB, and z1
            # gains ~0.5us of slack against its pos-10 deadline.
            Z0A, Z0B = 616, 1232
            # ACT ring: wt head, z0 middle, remaining wt, y-planes.
            wt_dma(0, 4)
            p_dma(nc.scalar, zplanes, xz_d, 0, Z0A, Z0B)
            wt_dma(4, 20)
            p_dma(nc.scalar, yplanes, xy_d, 0)
            wt_dma(20, 44)
            p_dma(nc.scalar, yplanes, xy_d, 1)
            wt_dma(44, 68)
            for i in range(2, PX):
                p_dma(nc.scalar, yplanes, xy_d, i)
            # SP ring: z0 head/tail pieces, z1-z4, then x/z by deadline.
            p_dma(nc.sync, zplanes, xz_d, 0, 0, Z0A)
            p_dma(nc.sync, zplanes, xz_d, 0, Z0B, PLANE)
            for i in (1, 2, 3, 4):
                p_dma(nc.sync, zplanes, xz_d, i)
            for kind, i in (("x", 0), ("x", 2), ("z", 5), ("x", 4), ("z", 6),
                            ("x", 1), ("x", 3), ("x", 5)):
                p_dma(nc.sync, zplanes if kind == "z" else xplanes,
                      xz_d if kind == "z" else xx_d, i)
            for i in range(7, PX):
                p_dma(nc.sync, zplanes, xz_d, i)
            for i in range(6, PX):
                p_dma(nc.sync, xplanes, xx_d, i)

            def plane(kind, i):
                g = {"z": zplanes, "y": yplanes, "x": xplanes}[kind][i]
                return g[:].rearrange("p (y z) -> p y z", y=PY)

            def unit_src(u, px, ty):
                if u[0] == "z":
                    _, dx, dy, zg = u
                    kind, dyy, zo = "z", dy, 2 * zg
                elif u[0] == "y":
                    _, dx, yg, _ = u
                    kind, dyy, zo = "y", 2 * yg, 4
                else:
                    _, xg, _, _ = u
                    dx, kind, dyy, zo = 2 * xg, "x", 4, 4
                yb = ty * 10 + dyy
                return plane(kind, px + dx)[:, yb:yb + 10, zo:zo + 40]

            nu = len(U63)
            for px in range(10):
                stage = spool.tile([128, 1600], mybir.dt.float32, name="stage", tag="stage")
                last = px == 9
                if not last:
                    # ty-innermost: 4 matmuls share one weight load
                    pss = [qpool.tile([128, 400], mybir.dt.float32, name=f"ps{t}", tag=f"ps{t}")
                           for t in range(4)]
                    first = [True, True]
                    for pos, ui in enumerate(U63):
                        grp = pos % 2
                        for ty in range(4):
                            nc.tensor.matmul(
                                pss[ty][grp * 64:(grp + 1) * 64, :],
                                wt_sb[:, ui * DIM:(ui + 1) * DIM],
                                unit_src(UNITS[ui], px, ty),
                                start=first[grp],
                                stop=(pos >= nu - 2),
                                tile_position=(0, grp * 64),
                            )
                        first[grp] = False
                    for ty in range(4):
                        nc.vector.tensor_copy(stage[:, ty * 400:(ty + 1) * 400], pss[ty][:])
                        nc.scalar.dma_start(
                            y_d.ap()[:, px * 1600 + ty * 400:px * 1600 + (ty + 1) * 400],
                            stage[:, ty * 400:(ty + 1) * 400])
                else:
                    # last plane: per-ty accumulation so only the final
                    # quarter's evacuation is exposed at the kernel tail;
                    # the final quarter itself runs as 280+120-col chains
                    # so most of it drains while the 120-col piece computes
                    for ty in range(4):
                        halves = ((0, 400),) if ty < 3 else ((0, 280), (280, 400))
                        for h0, h1 in halves:
                            # the 120-col final piece gets its own PSUM tile
                            # (tag ps1 is free again) so its first matmuls
                            # don't wait on the 280-col piece's evacuation
                            tag = f"ps{ty}" if h0 == 0 else "ps1"
                            ps = qpool.tile([128, h1 - h0], mybir.dt.float32,
                                            name=f"ps{ty}_{h0}", tag=tag)
                            yr0, nrow = h0 // 40, (h1 - h0) // 40
                            first = [True, True]
                            for pos, ui in enumerate(U63):
                                grp = pos % 2
                                src = unit_src(UNITS[ui], px, ty)
                                nc.tensor.matmul(
                                    ps[grp * 64:(grp + 1) * 64, :],
                                    wt_sb[:, ui * DIM:(ui + 1) * DIM],
                                    src[:, yr0:yr0 + nrow, :],
                                    start=first[grp],
                                    stop=(pos >= nu - 2),
                                    tile_position=(0, grp * 64),
                                )
                                first[grp] = False
                            nc.vector.tensor_copy(stage[:, ty * 400 + h0:ty * 400 + h1],
                                                  ps[:])
                            # final piece goes out on the idle SP ring so the
                            # ACT queue's end-of-program drain overlaps it
                            ring = nc.sync if h0 > 0 else nc.scalar
                            ring.dma_start(
                                y_d.ap()[:, px * 1600 + ty * 400 + h0:px * 1600 + ty * 400 + h1],
                                stage[:, ty * 400 + h0:ty * 400 + h1])

    orig = nc.to_json_bytes
    nc.to_json_bytes = functools.wraps(orig)(lambda: _split_sync_waits_json(orig()))
    return nc


def kernel(x, linear_weight, weight, _trace=False):
    x = np.asarray(x, np.float32)
    k = _build_tp_kernel(np.asarray(linear_weight), np.asarray(weight))
    wt = _pack_weights(k)

    in_maps = []
    for core in range(8):
        b, cx = divmod(core, 4)
        xz, xy, xx = _build_slabs(x[b], cx)
        in_maps.append({"xz": xz, "xy": xy, "xx": xx, "wt": wt})

    nc = _build_program()
    res = run_bass_kernel_spmd(nc, in_maps, core_ids=list(range(8)), trace=_trace)

    y = np.empty((2, 40, 40, 40, DIM), np.float32)
    for core in range(8):
        b, cx = divmod(core, 4)
        yc = res.results[core]["y"]
        s = (yc[:64] + yc[64:]).reshape(DIM, 10, 4, 10, 40)
        y[b, cx * 10:(cx + 1) * 10] = s.transpose(1, 2, 3, 4, 0).reshape(10, 40, 40, DIM)
    if _trace:
        kernel.last_results = res
    return y



# revision 14
# speedup vs baseline: 1.0084x; 1.0032x over previous
"""Trainium2 Bass kernel for nn_Convolution_77111842832763.

3D conv 5x5x5 SAME, 64->64 channels, input [2,40,40,40,64] fp32, plus an
irrep-wise linear self-connection (folded into the conv's center tap).

Strategy (8 NeuronCores, data-parallel):
  - Shard: core = batch(2) x x-chunk(4); each core computes a [10,40,40,64]
    output slab from a zero-padded [14,44,44,64] input slab (halo 2).
  - Host builds the 5^3 x 64 x 64 tensor-product kernel exactly (float64),
    folds the self-connection into the center tap, and packs per-tap weight
    blocks; weights/slabs are cast to fp16 on host (device matmuls run fp16
    with fp32 PSUM accumulation; measured end-to-end rel-err ~2.9e-4).
  - Device: TWO channel-major slab copies in SBUF [128, 14*44*44]:
      slab_z: partitions 0-63 = slab, 64-127 = slab shifted +1 z-voxel
      slab_y: partitions 0-63 = slab, 64-127 = slab shifted +1 y-row
    K=128 packing: the 125 taps become 63 matmul units per output tile:
      50 z-pair units  (dx,dy, dz in {(0,1),(2,3)})        -> slab_z
      10 y-pair units  (dx, dz=4 slice, dy in {(0,1),(2,3)}) -> slab_y
       3 x-pair units  (dy=4,dz=4 column, dx in {(0,1),(2,3),(4,zero)}) -> slab_x
  - Units alternate between PE column groups 0-63/64-127 (2x column
    tiling) accumulating into psum[0:64]/psum[64:128]; the two partial
    sums are DMA'd out separately and added on host.
  - Output tile = one x-plane quarter: 10 y-rows x 40 z = 400 voxels
    (moving free dim 400, one PSUM bank).  40 tiles per core.
  - DMA: z/x-copy planes on the SP HWDGE ring, weights/y-planes/outputs
    on the ACT ring (parallel rings), each ring ordered by consumption
    deadline; z-plane 0 is split across both rings so the first matmul's
    gate lands ~2x sooner.  A JSON post-pass splits multi-wait
    instructions (this walrus build allows one sync wait per instruction).
"""

import functools
import json
import math
from contextlib import ExitStack

import numpy as np

import concourse.bass as bass
import concourse.mybir as mybir
import concourse.tile as tile
from concourse.bass_utils import run_bass_kernel_spmd

MUL = 16
DIM = 64
NB = 8
PX, PY, PZ = 14, 44, 44          # padded slab dims
PLANE = PY * PZ                   # 1936
SLAB = PX * PLANE                 # 27104
SLAB_PAD = SLAB + 48              # tail zeros so the +1z and +44y views stay in-bounds
# unit list: ("z", dx, dy, zg) -> taps (dx,dy,2*zg)+(dx,dy,2*zg+1) via slab_z
#            ("y", dx, yg)     -> taps (dx,2*yg,4)+(dx,2*yg+1,4)   via slab_y
#            ("x", xg)         -> taps (2*xg,4,4)+(2*xg+1,4,4)     via slab_x
# The weight table holds all 68 unit blocks; every output plane runs the
# 63-unit scheme (the input DMA order streams the x-copy planes early
# enough that even plane 0's three x-pair units arrive in time).
UNITS = [("z", dx, dy, zg) for dx in range(5) for dy in range(5) for zg in range(2)]
UNITS += [("y", dx, yg, 0) for dx in range(5) for yg in range(3)]
UNITS += [("x", xg, 0, 0) for xg in range(3)]
NU = len(UNITS)                   # 68 weight blocks
U63 = list(range(50)) + [50 + 3 * dx + yg for dx in range(5) for yg in range(2)] \
    + [65, 66, 67]                                         # z50 + y10 + x3


def _split_sync_waits_json(raw: bytes) -> bytes:
    """Hoist all but the last sync wait of each instruction onto preceding
    same-engine EventSemaphore instructions (engines execute in order, so
    this is semantically identical)."""
    m = json.loads(raw)
    ctr = 0
    for fn in m.get("functions", []):
        for blk in fn.get("blocks", []):
            out = []
            for inst in blk.get("instructions", []):
                si = inst.get("sync_info")
                ow = (si or {}).get("on_wait") or []
                if len(ow) > 1:
                    for w in ow[:-1]:
                        ctr += 1
                        out.append({
                            "debug": inst.get("debug", 0),
                            "engine": inst["engine"],
                            "ins": [],
                            "outs": [],
                            "name": f"SWX-{ctr}",
                            "opcode": "EventSemaphore",
                            "sync_info": {"on_update": [], "on_wait": [w]},
                        })
                    si["on_wait"] = [ow[-1]]
                out.append(inst)
            blk["instructions"] = out
    return json.dumps(m).encode()


def _build_tp_kernel(linear_weight: np.ndarray, weight: np.ndarray) -> np.ndarray:
    """Mirror reference.py's CG kernel construction in float64.
    Returns k[5,5,5,64,64] ([dx,dy,dz,in,out]) with the self-connection
    folded into the center tap."""
    lw = linear_weight.astype(np.float64)
    w8 = weight.astype(np.float64)
    ax = np.arange(-2.0, 3.0)
    gx, gy, gz = np.meshgrid(ax, ax, ax, indexing="ij")
    lattice = np.stack([gx, gy, gz], axis=-1)            # [5,5,5,3]
    rad = np.linalg.norm(lattice, axis=-1)
    values = np.linspace(0.0, 2.5, NB + 2)[1:-1]
    step = 2.5 / (NB + 1)
    diff = (rad[..., None] - values) / step
    den = np.maximum(1.0 - diff * diff, 1e-9)
    emb = np.where(np.abs(diff) < 1.0, 1.14136 * np.exp(2.0 - 1.0 / den), 0.0)
    n = rad[..., None]
    unit = np.where(n > 0, lattice / np.where(n > 0, n, 1.0), 0.0)
    sh = np.concatenate([np.ones((5, 5, 5, 1)), math.sqrt(3.0) * unit], -1)
    L = 125
    w = (emb.reshape(L, NB) @ w8) / float(L)             # [125, 1024]
    W = w.reshape(L, 4, MUL, MUL)
    shf = sh.reshape(L, 4)
    y0, y1 = shf[:, 0], shf[:, 1:4]
    a = 1.0 / math.sqrt(2.0 * MUL)
    eye3 = np.eye(3)
    Rss = a * W[:, 0] * y0[:, None, None]
    Rsv = a * np.einsum("luw,lm->luwm", W[:, 1], y1).reshape(L, MUL, 3 * MUL)
    Rvv = a * np.einsum("luw,l,mn->lumwn", W[:, 2], y0, eye3).reshape(L, 3 * MUL, 3 * MUL)
    Rvs = (a / math.sqrt(3.0)) * np.einsum("luw,lm->lumw", W[:, 3], y1).reshape(L, 3 * MUL, MUL)
    k = np.concatenate(
        [np.concatenate([Rss, Rsv], -1), np.concatenate([Rvs, Rvv], -1)], 1
    ).reshape(5, 5, 5, DIM, DIM)
    # self-connection: irrep-wise linear, folded into center tap
    Wl = lw.reshape(2, MUL, MUL) / math.sqrt(MUL)
    sc = np.zeros((DIM, DIM))
    sc[:MUL, :MUL] = Wl[0]
    for m in range(3):
        idx = MUL + np.arange(MUL) * 3 + m
        sc[np.ix_(idx, idx)] = Wl[1]
    k = k.copy()
    k[2, 2, 2] += sc
    return k


def _pack_weights(k: np.ndarray) -> np.ndarray:
    """[128, 65*64] fp16 per-unit weight blocks (rows 64-127 = paired tap,
    zeros when unpaired)."""
    Wp = np.zeros((128, NU * DIM), np.float64)
    for ui, u in enumerate(UNITS):
        s = ui * DIM
        if u[0] == "z":
            _, dx, dy, zg = u
            Wp[0:64, s:s + DIM] = k[dx, dy, 2 * zg]
            Wp[64:128, s:s + DIM] = k[dx, dy, 2 * zg + 1]
        elif u[0] == "y":
            _, dx, yg, _ = u
            Wp[0:64, s:s + DIM] = k[dx, 2 * yg, 4]
            if 2 * yg + 1 < 5:
                Wp[64:128, s:s + DIM] = k[dx, 2 * yg + 1, 4]
        else:
            _, xg, _, _ = u
            Wp[0:64, s:s + DIM] = k[2 * xg, 4, 4]
            if 2 * xg + 1 < 5:
                Wp[64:128, s:s + DIM] = k[2 * xg + 1, 4, 4]
    return Wp.astype(np.float16)


def _build_slabs(xb: np.ndarray, cx: int):
    """Channel-major zero-padded fp16 slab copies for x-chunk cx of
    batch-slice xb [40,40,40,64]:
      xz [128, SLAB_PAD]: rows 0-63 slab, rows 64-127 slab shifted +1 z-voxel
      xy [128, SLAB_PAD]: rows 0-63 slab, rows 64-127 slab shifted +1 y-row
      xx [128, SLAB_PAD]: rows 0-63 slab, rows 64-127 slab shifted +1 x-plane
    (full-128-partition DMA runs at twice the rate of a 64-partition one)."""
    pad = np.zeros((PX, PY, PZ, DIM), np.float32)
    x0 = cx * 10 - 2
    lo, hi = max(0, x0), min(40, x0 + PX)
    pad[lo - x0:hi - x0, 2:42, 2:42, :] = xb[lo:hi]
    xs = np.zeros((DIM, SLAB_PAD), np.float16)
    xs[:, :SLAB] = np.ascontiguousarray(
        pad.transpose(3, 0, 1, 2)).reshape(DIM, SLAB).astype(np.float16)
    xz = np.zeros((128, SLAB_PAD), np.float16)
    xy = np.zeros((128, SLAB_PAD), np.float16)
    xx = np.zeros((128, SLAB_PAD), np.float16)
    for a in (xz, xy, xx):
        a[:DIM] = xs
    xz[DIM:, :SLAB_PAD - 1] = xs[:, 1:]
    xy[DIM:, :SLAB_PAD - PY] = xs[:, PY:]
    xx[DIM:, :SLAB_PAD - PLANE] = xs[:, PLANE:]
    return xz, xy, xx


def _build_program():
    nc = bass.Bass("TRN2", target_bir_lowering=False, debug=False)
    xz_d = nc.dram_tensor("xz", [128, SLAB_PAD], mybir.dt.float16, kind="ExternalInput")
    xy_d = nc.dram_tensor("xy", [128, SLAB_PAD], mybir.dt.float16, kind="ExternalInput")
    xx_d = nc.dram_tensor("xx", [128, SLAB_PAD], mybir.dt.float16, kind="ExternalInput")
    wt_d = nc.dram_tensor("wt", [128, NU * DIM], mybir.dt.float16, kind="ExternalInput")
    y_d = nc.dram_tensor("y", [128, 16000], mybir.dt.float32, kind="ExternalOutput")

    with tile.TileContext(nc) as tc:
        with ExitStack() as ctx:
            wpool = ctx.enter_context(tc.tile_pool(name="wts", bufs=1))
            ppool = ctx.enter_context(tc.tile_pool(name="planes", bufs=1))
            spool = ctx.enter_context(tc.tile_pool(name="stage", bufs=4))
            qpool = ctx.enter_context(tc.tile_pool(name="psum", bufs=2, space="PSUM"))

            # plane tiles first, DMAs issued below in deadline order
            wt_sb = wpool.tile([128, NU * DIM], mybir.dt.float16)
            zplanes = [ppool.tile([128, PLANE], mybir.dt.float16, name=f"pz{i}", tag=f"pz{i}")
                       for i in range(PX)]
            yplanes = [ppool.tile([128, PLANE], mybir.dt.float16, name=f"py{i}", tag=f"py{i}")
                       for i in range(PX)]
            xplanes = [ppool.tile([128, PLANE], mybir.dt.float16, name=f"px{i}", tag=f"px{i}")
                       for i in range(PX)]

            def wt_dma(lo, hi):
                nc.scalar.dma_start(wt_sb[:, lo * DIM:hi * DIM],
                                    wt_d.ap()[:, lo * DIM:hi * DIM])

            def p_dma(eng, tiles, dram, i, c0=0, c1=PLANE):
                o = i * PLANE
                eng.dma_start(tiles[i][:, c0:c1], dram.ap()[:, o + c0:o + c1])

            # Issue order = HW service order per ring; the two rings run in
            # parallel.  The first matmul gates on wt units 0-3 + z-plane 0,
            # so z0 is split across BOTH rings (2x bandwidth) and the first
            # weight chunk is tiny.  Later planes are ordered by the time
            # the unit stream consumes them (x0/x2/x4 before plane 0's three
            # x-pair units, etc.).
            # ACT ring: z0 back half, wt head, remaining wt, y-planes.
            p_dma(nc.scalar, zplanes, xz_d, 0, PLANE // 2, PLANE)
            wt_dma(0, 4)
            wt_dma(4, 20)
            p_dma(nc.scalar, yplanes, xy_d, 0)
            wt_dma(20, 44)
            p_dma(nc.scalar, yplanes, xy_d, 1)
            wt_dma(44, 68)
            for i in range(2, PX):
                p_dma(nc.scalar, yplanes, xy_d, i)
            # SP ring: z0 front half, z1-z4, then x/z interleaved by deadline.
            p_dma(nc.sync, zplanes, xz_d, 0, 0, PLANE // 2)
            for i in (1, 2, 3, 4):
                p_dma(nc.sync, zplanes, xz_d, i)
            for kind, i in (("x", 0), ("x", 2), ("z", 5), ("x", 4), ("z", 6),
                            ("x", 1), ("x", 3), ("x", 5)):
                p_dma(nc.sync, zplanes if kind == "z" else xplanes,
                      xz_d if kind == "z" else xx_d, i)
            for i in range(7, PX):
                p_dma(nc.sync, zplanes, xz_d, i)
            for i in range(6, PX):
                p_dma(nc.sync, xplanes, xx_d, i)

            def plane(kind, i):
                g = {"z": zplanes, "y": yplanes, "x": xplanes}[kind][i]
                return g[:].rearrange("p (y z) -> p y z", y=PY)

            def unit_src(u, px, ty):
                if u[0] == "z":
                    _, dx, dy, zg = u
                    kind, dyy, zo = "z", dy, 2 * zg
                elif u[0] == "y":
                    _, dx, yg, _ = u
                    kind, dyy, zo = "y", 2 * yg, 4
                else:
                    _, xg, _, _ = u
                    dx, kind, dyy, zo = 2 * xg, "x", 4, 4
                yb = ty * 10 + dyy
                return plane(kind, px + dx)[:, yb:yb + 10, zo:zo + 40]

            nu = len(U63)
            for px in range(10):
                stage = spool.tile([128, 1600], mybir.dt.float32, name="stage", tag="stage")
                last = px == 9
                if not last:
                    # ty-innermost: 4 matmuls share one weight load
                    pss = [qpool.tile([128, 400], mybir.dt.float32, name=f"ps{t}", tag=f"ps{t}")
                           for t in range(4)]
                    first = [True, True]
                    for pos, ui in enumerate(U63):
                        grp = pos % 2
                        for ty in range(4):
                            nc.tensor.matmul(
                                pss[ty][grp * 64:(grp + 1) * 64, :],
                                wt_sb[:, ui * DIM:(ui + 1) * DIM],
                                unit_src(UNITS[ui], px, ty),
                                start=first[grp],
                                stop=(pos >= nu - 2),
                                tile_position=(0, grp * 64),
                            )
                        first[grp] = False
                    for ty in range(4):
                        nc.vector.tensor_copy(stage[:, ty * 400:(ty + 1) * 400], pss[ty][:])
                        nc.scalar.dma_start(
                            y_d.ap()[:, px * 1600 + ty * 400:px * 1600 + (ty + 1) * 400],
                            stage[:, ty * 400:(ty + 1) * 400])
                else:
                    # last plane: per-ty accumulation so only the final
                    # quarter's evacuation is exposed at the kernel tail;
                    # the final quarter itself runs as 280+120-col chains
                    # so most of it drains while the 120-col piece computes
                    for ty in range(4):
                        halves = ((0, 400),) if ty < 3 else ((0, 280), (280, 400))
                        for h0, h1 in halves:
                            # the 120-col final piece gets its own PSUM tile
                            # (tag ps1 is free again) so its first matmuls
                            # don't wait on the 280-col piece's evacuation
                            tag = f"ps{ty}" if h0 == 0 else "ps1"
                            ps = qpool.tile([128, h1 - h0], mybir.dt.float32,
                                            name=f"ps{ty}_{h0}", tag=tag)
                            yr0, nrow = h0 // 40, (h1 - h0) // 40
                            first = [True, True]
                            for pos, ui in enumerate(U63):
                                grp = pos % 2
                                src = unit_src(UNITS[ui], px, ty)
                                nc.tensor.matmul(
                                    ps[grp * 64:(grp + 1) * 64, :],
                                    wt_sb[:, ui * DIM:(ui + 1) * DIM],
                                    src[:, yr0:yr0 + nrow, :],
                                    start=first[grp],
                                    stop=(pos >= nu - 2),
                                    tile_position=(0, grp * 64),
                                )
                                first[grp] = False
                            nc.vector.tensor_copy(stage[:, ty * 400 + h0:ty * 400 + h1],
                                                  ps[:])
                            # final piece goes out on the idle SP ring so the
                            # ACT queue's end-of-program drain overlaps it
                            ring = nc.sync if h0 > 0 else nc.scalar
                            ring.dma_start(
                                y_d.ap()[:, px * 1600 + ty * 400 + h0:px * 1600 + ty * 400 + h1],
                                stage[:, ty * 400 + h0:ty * 400 + h1])

    orig = nc.to_json_bytes
    nc.to_json_bytes = functools.wraps(orig)(lambda: _split_sync_waits_json(orig()))
    return nc


def kernel(x, linear_weight, weight, _trace=False):
    x = np.asarray(x, np.float32)
    k = _build_tp_kernel(np.asarray(linear_weight), np.asarray(weight))
    wt = _pack_weights(k)

    in_maps = []
    for core in range(8):
        b, cx = divmod(core, 4)
        xz, xy, xx = _build_slabs(x[b], cx)
        in_maps.append({"xz": xz, "xy": xy, "xx": xx, "wt": wt})

    nc = _build_program()
    res = run_bass_kernel_spmd(nc, in_maps, core_ids=list(range(8)), trace=_trace)

    y = np.empty((2, 40, 40, 40, DIM), np.float32)
    for core in range(8):
        b, cx = divmod(core, 4)
        yc = res.results[core]["y"]
        s = (yc[:64] + yc[64:]).reshape(DIM, 10, 4, 10, 40)
        y[b, cx * 10:(cx + 1) * 10] = s.transpose(1, 2, 3, 4, 0).reshape(10, 40, 40, DIM)
    if _trace:
        kernel.last_results = res
    return y



# revision 15
# speedup vs baseline: 1.0100x; 1.0015x over previous
"""Trainium2 Bass kernel for nn_Convolution_77111842832763.

3D conv 5x5x5 SAME, 64->64 channels, input [2,40,40,40,64] fp32, plus an
irrep-wise linear self-connection (folded into the conv's center tap).

Strategy (8 NeuronCores, data-parallel):
  - Shard: core = batch(2) x x-chunk(4); each core computes a [10,40,40,64]
    output slab from a zero-padded [14,44,44,64] input slab (halo 2).
  - Host builds the 5^3 x 64 x 64 tensor-product kernel exactly (float64),
    folds the self-connection into the center tap, and packs per-tap weight
    blocks; weights/slabs are cast to fp16 on host (device matmuls run fp16
    with fp32 PSUM accumulation; measured end-to-end rel-err ~2.9e-4).
  - Device: TWO channel-major slab copies in SBUF [128, 14*44*44]:
      slab_z: partitions 0-63 = slab, 64-127 = slab shifted +1 z-voxel
      slab_y: partitions 0-63 = slab, 64-127 = slab shifted +1 y-row
    K=128 packing: the 125 taps become 63 matmul units per output tile:
      50 z-pair units  (dx,dy, dz in {(0,1),(2,3)})        -> slab_z
      10 y-pair units  (dx, dz=4 slice, dy in {(0,1),(2,3)}) -> slab_y
       3 x-pair units  (dy=4,dz=4 column, dx in {(0,1),(2,3),(4,zero)}) -> slab_x
  - Units alternate between PE column groups 0-63/64-127 (2x column
    tiling) accumulating into psum[0:64]/psum[64:128]; the two partial
    sums are DMA'd out separately and added on host.
  - Output tile = one x-plane quarter: 10 y-rows x 40 z = 400 voxels
    (moving free dim 400, one PSUM bank).  40 tiles per core.
  - DMA: z/x-copy planes on the SP HWDGE ring, weights/y-planes/outputs
    on the ACT ring (parallel rings), each ring ordered by consumption
    deadline; z-plane 0 is split across both rings so the first matmul's
    gate lands ~2x sooner.  A JSON post-pass splits multi-wait
    instructions (this walrus build allows one sync wait per instruction).
"""

import functools
import json
import math
from contextlib import ExitStack

import numpy as np

import concourse.bass as bass
import concourse.mybir as mybir
import concourse.tile as tile
from concourse.bass_utils import run_bass_kernel_spmd

MUL = 16
DIM = 64
NB = 8
PX, PY, PZ = 14, 44, 44          # padded slab dims
PLANE = PY * PZ                   # 1936
SLAB = PX * PLANE                 # 27104
SLAB_PAD = SLAB + 48              # tail zeros so the +1z and +44y views stay in-bounds
# unit list: ("z", dx, dy, zg) -> taps (dx,dy,2*zg)+(dx,dy,2*zg+1) via slab_z
#            ("y", dx, yg)     -> taps (dx,2*yg,4)+(dx,2*yg+1,4)   via slab_y
#            ("x", xg)         -> taps (2*xg,4,4)+(2*xg+1,4,4)     via slab_x
# The weight table holds all 68 unit blocks; every output plane runs the
# 63-unit scheme (the input DMA order streams the x-copy planes early
# enough that even plane 0's three x-pair units arrive in time).
UNITS = [("z", dx, dy, zg) for dx in range(5) for dy in range(5) for zg in range(2)]
UNITS += [("y", dx, yg, 0) for dx in range(5) for yg in range(3)]
UNITS += [("x", xg, 0, 0) for xg in range(3)]
NU = len(UNITS)                   # 68 weight blocks
U63 = list(range(50)) + [50 + 3 * dx + yg for dx in range(5) for yg in range(2)] \
    + [65, 66, 67]                                         # z50 + y10 + x3


def _split_sync_waits_json(raw: bytes) -> bytes:
    """Hoist all but the last sync wait of each instruction onto preceding
    same-engine EventSemaphore instructions (engines execute in order, so
    this is semantically identical)."""
    m = json.loads(raw)
    ctr = 0
    for fn in m.get("functions", []):
        for blk in fn.get("blocks", []):
            out = []
            for inst in blk.get("instructions", []):
                si = inst.get("sync_info")
                ow = (si or {}).get("on_wait") or []
                if len(ow) > 1:
                    for w in ow[:-1]:
                        ctr += 1
                        out.append({
                            "debug": inst.get("debug", 0),
                            "engine": inst["engine"],
                            "ins": [],
                            "outs": [],
                            "name": f"SWX-{ctr}",
                            "opcode": "EventSemaphore",
                            "sync_info": {"on_update": [], "on_wait": [w]},
                        })
                    si["on_wait"] = [ow[-1]]
                out.append(inst)
            blk["instructions"] = out
    return json.dumps(m).encode()


def _build_tp_kernel(linear_weight: np.ndarray, weight: np.ndarray) -> np.ndarray:
    """Mirror reference.py's CG kernel construction in float64.
    Returns k[5,5,5,64,64] ([dx,dy,dz,in,out]) with the self-connection
    folded into the center tap."""
    lw = linear_weight.astype(np.float64)
    w8 = weight.astype(np.float64)
    ax = np.arange(-2.0, 3.0)
    gx, gy, gz = np.meshgrid(ax, ax, ax, indexing="ij")
    lattice = np.stack([gx, gy, gz], axis=-1)            # [5,5,5,3]
    rad = np.linalg.norm(lattice, axis=-1)
    values = np.linspace(0.0, 2.5, NB + 2)[1:-1]
    step = 2.5 / (NB + 1)
    diff = (rad[..., None] - values) / step
    den = np.maximum(1.0 - diff * diff, 1e-9)
    emb = np.where(np.abs(diff) < 1.0, 1.14136 * np.exp(2.0 - 1.0 / den), 0.0)
    n = rad[..., None]
    unit = np.where(n > 0, lattice / np.where(n > 0, n, 1.0), 0.0)
    sh = np.concatenate([np.ones((5, 5, 5, 1)), math.sqrt(3.0) * unit], -1)
    L = 125
    w = (emb.reshape(L, NB) @ w8) / float(L)             # [125, 1024]
    W = w.reshape(L, 4, MUL, MUL)
    shf = sh.reshape(L, 4)
    y0, y1 = shf[:, 0], shf[:, 1:4]
    a = 1.0 / math.sqrt(2.0 * MUL)
    eye3 = np.eye(3)
    Rss = a * W[:, 0] * y0[:, None, None]
    Rsv = a * np.einsum("luw,lm->luwm", W[:, 1], y1).reshape(L, MUL, 3 * MUL)
    Rvv = a * np.einsum("luw,l,mn->lumwn", W[:, 2], y0, eye3).reshape(L, 3 * MUL, 3 * MUL)
    Rvs = (a / math.sqrt(3.0)) * np.einsum("luw,lm->lumw", W[:, 3], y1).reshape(L, 3 * MUL, MUL)
    k = np.concatenate(
        [np.concatenate([Rss, Rsv], -1), np.concatenate([Rvs, Rvv], -1)], 1
    ).reshape(5, 5, 5, DIM, DIM)
    # self-connection: irrep-wise linear, folded into center tap
    Wl = lw.reshape(2, MUL, MUL) / math.sqrt(MUL)
    sc = np.zeros((DIM, DIM))
    sc[:MUL, :MUL] = Wl[0]
    for m in range(3):
        idx = MUL + np.arange(MUL) * 3 + m
        sc[np.ix_(idx, idx)] = Wl[1]
    k = k.copy()
    k[2, 2, 2] += sc
    return k


def _pack_weights(k: np.ndarray) -> np.ndarray:
    """[128, 65*64] fp16 per-unit weight blocks (rows 64-127 = paired tap,
    zeros when unpaired)."""
    Wp = np.zeros((128, NU * DIM), np.float64)
    for ui, u in enumerate(UNITS):
        s = ui * DIM
        if u[0] == "z":
            _, dx, dy, zg = u
            Wp[0:64, s:s + DIM] = k[dx, dy, 2 * zg]
            Wp[64:128, s:s + DIM] = k[dx, dy, 2 * zg + 1]
        elif u[0] == "y":
            _, dx, yg, _ = u
            Wp[0:64, s:s + DIM] = k[dx, 2 * yg, 4]
            if 2 * yg + 1 < 5:
                Wp[64:128, s:s + DIM] = k[dx, 2 * yg + 1, 4]
        else:
            _, xg, _, _ = u
            Wp[0:64, s:s + DIM] = k[2 * xg, 4, 4]
            if 2 * xg + 1 < 5:
                Wp[64:128, s:s + DIM] = k[2 * xg + 1, 4, 4]
    return Wp.astype(np.float16)


def _build_slabs(xb: np.ndarray, cx: int):
    """Channel-major zero-padded fp16 slab copies for x-chunk cx of
    batch-slice xb [40,40,40,64]:
      xz [128, SLAB_PAD]: rows 0-63 slab, rows 64-127 slab shifted +1 z-voxel
      xy [128, SLAB_PAD]: rows 0-63 slab, rows 64-127 slab shifted +1 y-row
      xx [128, SLAB_PAD]: rows 0-63 slab, rows 64-127 slab shifted +1 x-plane
    (full-128-partition DMA runs at twice the rate of a 64-partition one)."""
    pad = np.zeros((PX, PY, PZ, DIM), np.float32)
    x0 = cx * 10 - 2
    lo, hi = max(0, x0), min(40, x0 + PX)
    pad[lo - x0:hi - x0, 2:42, 2:42, :] = xb[lo:hi]
    xs = np.zeros((DIM, SLAB_PAD), np.float16)
    xs[:, :SLAB] = np.ascontiguousarray(
        pad.transpose(3, 0, 1, 2)).reshape(DIM, SLAB).astype(np.float16)
    xz = np.zeros((128, SLAB_PAD), np.float16)
    xy = np.zeros((128, SLAB_PAD), np.float16)
    xx = np.zeros((128, SLAB_PAD), np.float16)
    for a in (xz, xy, xx):
        a[:DIM] = xs
    xz[DIM:, :SLAB_PAD - 1] = xs[:, 1:]
    xy[DIM:, :SLAB_PAD - PY] = xs[:, PY:]
    xx[DIM:, :SLAB_PAD - PLANE] = xs[:, PLANE:]
    return xz, xy, xx


def _build_program():
    nc = bass.Bass("TRN2", target_bir_lowering=False, debug=False)
    xz_d = nc.dram_tensor("xz", [128, SLAB_PAD], mybir.dt.float16, kind="ExternalInput")
    xy_d = nc.dram_tensor("xy", [128, SLAB_PAD], mybir.dt.float16, kind="ExternalInput")
    xx_d = nc.dram_tensor("xx", [128, SLAB_PAD], mybir.dt.float16, kind="ExternalInput")
    wt_d = nc.dram_tensor("wt", [128, NU * DIM], mybir.dt.float16, kind="ExternalInput")
    y_d = nc.dram_tensor("y", [128, 16000], mybir.dt.float32, kind="ExternalOutput")

    with tile.TileContext(nc) as tc:
        with ExitStack() as ctx:
            wpool = ctx.enter_context(tc.tile_pool(name="wts", bufs=1))
            ppool = ctx.enter_context(tc.tile_pool(name="planes", bufs=1))
            spool = ctx.enter_context(tc.tile_pool(name="stage", bufs=4))
            qpool = ctx.enter_context(tc.tile_pool(name="psum", bufs=2, space="PSUM"))

            # plane tiles first, DMAs issued below in deadline order
            wt_sb = wpool.tile([128, NU * DIM], mybir.dt.float16)
            zplanes = [ppool.tile([128, PLANE], mybir.dt.float16, name=f"pz{i}", tag=f"pz{i}")
                       for i in range(PX)]
            yplanes = [ppool.tile([128, PLANE], mybir.dt.float16, name=f"py{i}", tag=f"py{i}")
                       for i in range(PX)]
            xplanes = [ppool.tile([128, PLANE], mybir.dt.float16, name=f"px{i}", tag=f"px{i}")
                       for i in range(PX)]

            def wt_dma(lo, hi):
                nc.scalar.dma_start(wt_sb[:, lo * DIM:hi * DIM],
                                    wt_d.ap()[:, lo * DIM:hi * DIM])

            def p_dma(eng, tiles, dram, i, c0=0, c1=PLANE):
                o = i * PLANE
                eng.dma_start(tiles[i][:, c0:c1], dram.ap()[:, o + c0:o + c1])

            # Issue order = HW service order per ring; the two rings run in
            # parallel.  The first matmul gates on wt units 0-3 + z-plane 0,
            # so z0 is split across BOTH rings (2x bandwidth) and the first
            # weight chunk is tiny.  Later planes are ordered by the time
            # the unit stream consumes them (x0/x2/x4 before plane 0's three
            # x-pair units, etc.).
            # z0 goes as FOUR pieces (two per ring): consecutive dma_starts
            # land on distinct DMAHW queues and engine time is split per
            # active queue, so four queues double z0's share at the head.
            # ACT ring: z0 pieces b1/b2 around the wt head, wt, y-planes.
            p_dma(nc.scalar, zplanes, xz_d, 0, 484, 968)
            wt_dma(0, 4)
            p_dma(nc.scalar, zplanes, xz_d, 0, 1452, PLANE)
            wt_dma(4, 20)
            p_dma(nc.scalar, yplanes, xy_d, 0)
            wt_dma(20, 44)
            p_dma(nc.scalar, yplanes, xy_d, 1)
            wt_dma(44, 68)
            for i in range(2, PX):
                p_dma(nc.scalar, yplanes, xy_d, i)
            # SP ring: z0 pieces a1/a2, z1-z4, then x/z by deadline.
            p_dma(nc.sync, zplanes, xz_d, 0, 0, 484)
            p_dma(nc.sync, zplanes, xz_d, 0, 968, 1452)
            for i in (1, 2, 3, 4):
                p_dma(nc.sync, zplanes, xz_d, i)
            for kind, i in (("x", 0), ("x", 2), ("z", 5), ("x", 4), ("z", 6),
                            ("x", 1), ("x", 3), ("x", 5)):
                p_dma(nc.sync, zplanes if kind == "z" else xplanes,
                      xz_d if kind == "z" else xx_d, i)
            for i in range(7, PX):
                p_dma(nc.sync, zplanes, xz_d, i)
            for i in range(6, PX):
                p_dma(nc.sync, xplanes, xx_d, i)

            def plane(kind, i):
                g = {"z": zplanes, "y": yplanes, "x": xplanes}[kind][i]
                return g[:].rearrange("p (y z) -> p y z", y=PY)

            def unit_src(u, px, ty):
                if u[0] == "z":
                    _, dx, dy, zg = u
                    kind, dyy, zo = "z", dy, 2 * zg
                elif u[0] == "y":
                    _, dx, yg, _ = u
                    kind, dyy, zo = "y", 2 * yg, 4
                else:
                    _, xg, _, _ = u
                    dx, kind, dyy, zo = 2 * xg, "x", 4, 4
                yb = ty * 10 + dyy
                return plane(kind, px + dx)[:, yb:yb + 10, zo:zo + 40]

            nu = len(U63)
            for px in range(10):
                stage = spool.tile([128, 1600], mybir.dt.float32, name="stage", tag="stage")
                last = px == 9
                if not last:
                    # ty-innermost: 4 matmuls share one weight load
                    pss = [qpool.tile([128, 400], mybir.dt.float32, name=f"ps{t}", tag=f"ps{t}")
                           for t in range(4)]
                    first = [True, True]
                    for pos, ui in enumerate(U63):
                        grp = pos % 2
                        for ty in range(4):
                            nc.tensor.matmul(
                                pss[ty][grp * 64:(grp + 1) * 64, :],
                                wt_sb[:, ui * DIM:(ui + 1) * DIM],
                                unit_src(UNITS[ui], px, ty),
                                start=first[grp],
                                stop=(pos >= nu - 2),
                                tile_position=(0, grp * 64),
                            )
                        first[grp] = False
                    for ty in range(4):
                        nc.vector.tensor_copy(stage[:, ty * 400:(ty + 1) * 400], pss[ty][:])
                        nc.scalar.dma_start(
                            y_d.ap()[:, px * 1600 + ty * 400:px * 1600 + (ty + 1) * 400],
                            stage[:, ty * 400:(ty + 1) * 400])
                else:
                    # last plane: per-ty accumulation so only the final
                    # quarter's evacuation is exposed at the kernel tail;
                    # the final quarter itself runs as 280+120-col chains
                    # so most of it drains while the 120-col piece computes
                    for ty in range(4):
                        halves = ((0, 400),) if ty < 3 else ((0, 280), (280, 400))
                        for h0, h1 in halves:
                            # the 120-col final piece gets its own PSUM tile
                            # (tag ps1 is free again) so its first matmuls
                            # don't wait on the 280-col piece's evacuation
                            tag = f"ps{ty}" if h0 == 0 else "ps1"
                            ps = qpool.tile([128, h1 - h0], mybir.dt.float32,
                                            name=f"ps{ty}_{h0}", tag=tag)
                            yr0, nrow = h0 // 40, (h1 - h0) // 40
                            first = [True, True]
                            for pos, ui in enumerate(U63):
                                grp = pos % 2
                                src = unit_src(UNITS[ui], px, ty)
                                nc.tensor.matmul(
                                    ps[grp * 64:(grp + 1) * 64, :],
                                    wt_sb[:, ui * DIM:(ui + 1) * DIM],
                                    src[:, yr0:yr0 + nrow, :],
                                    start=first[grp],
                                    stop=(pos >= nu - 2),
                                    tile_position=(0, grp * 64),
                                )
                                first[grp] = False
                            nc.vector.tensor_copy(stage[:, ty * 400 + h0:ty * 400 + h1],
                                                  ps[:])
                            # final piece goes out on the idle SP ring so the
                            # ACT queue's end-of-program drain overlaps it
                            ring = nc.sync if h0 > 0 else nc.scalar
                            ring.dma_start(
                                y_d.ap()[:, px * 1600 + ty * 400 + h0:px * 1600 + ty * 400 + h1],
                                stage[:, ty * 400 + h0:ty * 400 + h1])

    orig = nc.to_json_bytes
    nc.to_json_bytes = functools.wraps(orig)(lambda: _split_sync_waits_json(orig()))
    return nc


def kernel(x, linear_weight, weight, _trace=False):
    x = np.asarray(x, np.float32)
    k = _build_tp_kernel(np.asarray(linear_weight), np.asarray(weight))
    wt = _pack_weights(k)

    in_maps = []
    for core in range(8):
        b, cx = divmod(core, 4)
        xz, xy, xx = _build_slabs(x[b], cx)
        in_maps.append({"xz": xz, "xy": xy, "xx": xx, "wt": wt})

    nc = _build_program()
    res = run_bass_kernel_spmd(nc, in_maps, core_ids=list(range(8)), trace=_trace)

    y = np.empty((2, 40, 40, 40, DIM), np.float32)
    for core in range(8):
        b, cx = divmod(core, 4)
        yc = res.results[core]["y"]
        s = (yc[:64] + yc[64:]).reshape(DIM, 10, 4, 10, 40)
        y[b, cx * 10:(cx + 1) * 10] = s.transpose(1, 2, 3, 4, 0).reshape(10, 40, 40, DIM)
    if _trace:
        kernel.last_results = res
    return y

